# revision 1
# baseline (speedup 1.0000x reference)
"""Trainium2 Bass kernel for MultiHeadAttentionRoPE.

Problem (hardcoded): B=2, S=2048, D=1024, H=16 heads, Dh=64, fp32 I/O.
    qkv = x @ w_qkv ; q,k -> RoPE ; causal attention ; out = ctx @ w_proj

Sharding: core = (batch, head-quad). Each of the 8 cores handles one of
the 2 batches and 4 of the 16 heads: it reads its batch's x (bf16,
transposed on host), its 4-head slice of w_qkv/w_proj, computes causal
attention for those heads and a *partial* projection output [S, D]; the
host sums the 4 partials per batch (the gather step of row-parallel TP).

All matmul operands are bf16 (fp32 PSUM accumulation); rel-err vs the
fp32 reference is ~6e-3 (tolerance 2e-2).

Per-core layout:
  - x fed transposed (d on partitions) so QKV projection produces qT/kT
    directly in (feature, token) layout for the scores matmul. Heads are
    packed in pairs per 128-partition tile (h_even rows 0:64, h_odd 64:128).
  - RoPE: rotate-half via partition-swapping SBUF->SBUF DMAs on a bf16
    staging tile, sin sign-folded on host; combine on DVE (bf16 4x mode).
  - scores are computed transposed (keys on partitions, queries free);
    exp runs on ACT (psum -> bf16 sbuf); the softmax denominator comes
    from a ones-column appended to each head's v block in the PV matmul.
  - v is transposed to natural (token, feature) layout via PE transposes.
  - causal masking: key-blocks strictly below the diagonal are unmasked;
    diagonal 128x128 blocks get a triangular 0/1 mask multiply and their
    fully-masked column prefix is skipped (matmul, exp, PV column-trimmed).
  - normalization: denominator row staged psum->sbuf (DVE, the custom
    recip misreads PSUM), fast approximate reciprocal (custom DVE op),
    partition-broadcast on GPSIMD, applied in the psum->sbuf ctx
    downcast (DVE). PV accumulators are split into one-bank column
    halves so PSUM slots free early at round boundaries.
  - projection per query chunk; partial outputs staged in sbuf (psum
    drain split across DVE and ACT) and stored from the idle SP queue;
    x for the second token group prefetches via Pool SWDGE.
"""

import functools
import os
import sys

import numpy as np

sys.path.insert(0, "/opt/trn_rl_repo")

# ---- problem constants (must match reference.py) ----
B = 2
S = 2048
D = 1024
H = 16
Dh = 64
N_CORES = 8
HPC = 4                     # heads per core
KC = D // 128               # contraction chunks = 8
TCH = 1024                  # token chunk (stage 1 groups and query blocks)
NG = S // TCH               # 2 groups
NKB = S // 128              # 16 key blocks
ROPE_BASE = 10000.0
SCALE = 1.0 / 8.0           # 1/sqrt(Dh)


def _build_program(loop_n=1, phases="all", opts=""):
    import concourse.bass as bass
    opts = set(opts.split(",")) if opts else set()
    import concourse.mybir as mybir
    import concourse.tile as tile
    from concourse import bacc
    from contextlib import ExitStack

    FP = mybir.dt.float32
    BF = mybir.dt.bfloat16
    FPR = mybir.dt.float32r
    EXP = mybir.ActivationFunctionType.Exp
    R = lambda ap: ap.bitcast(FPR)

    nc = bacc.Bacc("TRN2", target_bir_lowering=False, debug=False)

    xt_d = nc.dram_tensor("xt", [KC, 128, S], BF, kind="ExternalInput").ap()
    wqk_d = nc.dram_tensor("wqk", [KC, 128, 6 * 128], BF, kind="ExternalInput").ap()
    wproj_d = nc.dram_tensor("wproj", [2, 128, D], BF, kind="ExternalInput").ap()
    cos_d = nc.dram_tensor("cost", [128, S], BF, kind="ExternalInput").ap()
    sin_d = nc.dram_tensor("sint", [128, S], BF, kind="ExternalInput").ap()
    tri_d = nc.dram_tensor("tri", [128, 128], BF, kind="ExternalInput").ap()
    ones_d = nc.dram_tensor("onesc", [128, NKB, 65 * HPC], BF, kind="ExternalInput").ap()
    onesr_d = nc.dram_tensor("onesr", [1, 64], FP, kind="ExternalInput").ap()
    ident_d = nc.dram_tensor("ident", [128, 128], BF, kind="ExternalInput").ap()
    out_d = nc.dram_tensor("out", [S, D], BF, kind="ExternalOutput").ap()
    dbg = phases == "dbg"
    if dbg:
        qt_dbg = nc.dram_tensor("qt_dbg", [2, 128, S], BF, kind="ExternalOutput").ap()
        kt_dbg = nc.dram_tensor("kt_dbg", [2, 128, S], BF, kind="ExternalOutput").ap()
        vt_dbg = nc.dram_tensor("vt_dbg", [128, NKB, 65 * HPC], BF, kind="ExternalOutput").ap()
        ctx_dbg = nc.dram_tensor("ctx_dbg", [2, 128, S], BF, kind="ExternalOutput").ap()

    with tile.TileContext(nc) as tc, ExitStack() as ctx:
        consts = ctx.enter_context(tc.tile_pool(name="consts", bufs=1))
        store = ctx.enter_context(tc.tile_pool(name="store", bufs=1))
        xt_pool = ctx.enter_context(tc.tile_pool(name="xt_pool", bufs=2))
        rt_pool = ctx.enter_context(tc.tile_pool(name="rt_pool", bufs=2))
        p_pool = ctx.enter_context(tc.tile_pool(name="p_pool", bufs=6))
        nrm_pool = ctx.enter_context(tc.tile_pool(name="nrm_pool", bufs=2))
        ob_pool = ctx.enter_context(tc.tile_pool(name="ob_pool", bufs=2))
        # PSUM: 8 banks of [128, 2KB]. ps_a: rotating [128,1024]f32 (2 banks
        # each, 4 total) for qkv / scores / pbc / proj. ps_b: 4 banks for the
        # v-transpose scratch in stage 1 and the pv accumulators in attention.
        ps_a = ctx.enter_context(tc.tile_pool(name="ps_a", bufs=2, space="PSUM"))
        ps_b = ctx.enter_context(tc.tile_pool(name="ps_b", bufs=4, space="PSUM"))

        # ---- constants ----
        wqk_sb = consts.tile([128, KC, 6 * 128], BF, name="wqk_sb")
        ident_sb = consts.tile([128, 128], BF, name="ident_sb")
        wproj_sb = consts.tile([128, 2, D], BF, name="wproj_sb")
        cos_sb = consts.tile([128, S], BF, name="cos_sb")
        sin_sb = consts.tile([128, S], BF, name="sin_sb")
        tri_sb = consts.tile([128, 128], BF, name="tri_sb")
        onesr_sb = consts.tile([1, 64], FP, name="onesr_sb")

        # ---- persistent per-core storage ----
        # qT/kT chunk c holds heads (2c, 2c+1): rows [h dh0..63 | h' dh0..63]
        qT = {c: store.tile([128, S], BF, name=f"qT_{c}", tag=f"qT_{c}") for c in range(2)}
        kT = {c: store.tile([128, S], BF, name=f"kT_{c}", tag=f"kT_{c}") for c in range(2)}
        ctxT = {c: store.tile([128, S], BF, name=f"ctxT_{c}", tag=f"ctxT_{c}") for c in range(2)}
        # vt: per key block kb, cols [v_h0 |1| v_h1 |1| v_h2 |1| v_h3 |1]
        vt = store.tile([128, NKB, 65 * HPC], BF, name="vt", tag="vt")

        def late_consts():
            with tc.tile_wait_until(0.004):
                nc.sync.dma_start(cos_sb, cos_d)
                nc.sync.dma_start(sin_sb, sin_d)
            with tc.tile_wait_until(0.006):
                # whole-tile ones init: the interleave copies overwrite the v
                # columns; col 64 of each 65-block stays 1 (the denominator
                # column). Full-tile write gives unambiguous ordering vs the
                # PV matmul reads (a strided column DMA racing with the
                # interleave writes corrupted adjacent values on HW).
                nc.sync.dma_start(vt, ones_d)
            with tc.tile_wait_until(0.008):
                nc.gpsimd.dma_start(ident_sb, ident_d)
                nc.gpsimd.dma_start(tri_sb, tri_d)
                nc.sync.dma_start(R(onesr_sb), R(onesr_d))
                for i in range(2):
                    nc.gpsimd.dma_start(wproj_sb[:, i, :], wproj_d[i])

        xtiles = {}

        def stage1(g, half):
            """QKV^T projection + RoPE + v natural layout for token group g,
            head pair `half` (0: heads 0,1 / 1: heads 2,3). Emission order is
            k, q, v so attention round `half` of query block g can follow
            immediately."""
            tsl = slice(g * TCH, (g + 1) * TCH)
            if half == 0:
                xtile = xt_pool.tile([128, KC, TCH], BF, name="xtile", tag="xt")
                xtiles[g] = xtile
                for kc in range(KC):
                    if g == 0:
                        # alternate both startup streams across the HWDGE and
                        # Pool SWDGE queues so descriptor generation overlaps
                        if kc % 2 == 0:
                            nc.sync.dma_start(wqk_sb[:, kc, :], wqk_d[kc])
                            nc.gpsimd.dma_start(xtile[:, kc, :], xt_d[kc, :, tsl])
                        else:
                            nc.gpsimd.dma_start(wqk_sb[:, kc, :], wqk_d[kc])
                            nc.sync.dma_start(xtile[:, kc, :], xt_d[kc, :, tsl])
                    else:
                        # prefetch via Pool SWDGE (keeps HWDGE free for the
                        # RoPE swaps), delayed past the startup-critical loads
                        with tc.tile_wait_until(0.012):
                            nc.gpsimd.dma_start(xtile[:, kc, :], xt_d[kc, :, tsl])
                if g == 0:
                    late_consts()
            xtile = xtiles[g]
            rtile = rt_pool.tile([128, 2, TCH], BF, name="rtile", tag="rt")
            qs = rt_pool.tile([128, 2, TCH], BF, name="qs", tag="qs")
            # i: 0 = k-pair, 1 = q-pair, 2 = v-pair; fc indexes wqk q|k|v cols
            for i, fc in enumerate((2 + half, 0 + half, 4 + half)):
                psq = ps_a.tile([128, TCH], FP, name="psq", tag="ps_a")
                for kc in range(KC):
                    for hh in range(2):  # moving operand max 512 cols
                        nc.tensor.matmul(
                            psq[:, hh * 512:(hh + 1) * 512],
                            lhsT=wqk_sb[:, kc, fc * 128:(fc + 1) * 128],
                            rhs=xtile[:, kc, hh * 512:(hh + 1) * 512],
                            start=(kc == 0),
                            stop=(kc == KC - 1),
                        )
                if i < 2:
                    # stage q/k in bf16 for the rotate-half partition swap
                    # (GPSIMD cannot read PSUM; avoid ACT in the window that
                    # overlaps attention exps)
                    if i == 0 and not (g == 1 and half == 0):
                        nc.scalar.copy(rtile[:, i, :], psq)
                    else:
                        nc.vector.tensor_copy(out=rtile[:, i, :], in_=psq)
                    if i == 1:
                        # alternate the 4 range-swaps across HWDGE and Pool
                        # SWDGE so their generation slots overlap
                        for j, (d0, s0) in enumerate(
                            ((0, 32), (32, 0), (64, 96), (96, 64))
                        ):
                            eng = nc.sync if j % 2 == 0 else nc.gpsimd
                            eng.dma_start(
                                qs[d0:d0 + 32, :, :], rtile[s0:s0 + 32, :, :]
                            )
                        for j, dest in enumerate((kT[half], qT[half])):
                            t1 = rt_pool.tile([128, TCH], BF, name="t1", tag="t1")
                            nc.vector.tensor_mul(out=t1, in0=qs[:, j, :], in1=sin_sb[:, tsl])
                            nc.vector.tensor_mul(out=dest[:, tsl], in0=rtile[:, j, :], in1=cos_sb[:, tsl])
                            nc.vector.tensor_add(out=dest[:, tsl], in0=dest[:, tsl], in1=t1)
                else:
                    # v -> natural layout via PE transposes
                    vts = rt_pool.tile([128, TCH], BF, name="vts", tag="vts")
                    if g == 1 and half == 0:
                        nc.vector.tensor_copy(out=vts, in_=psq)
                    else:
                        nc.scalar.copy(vts, psq)
                    pv4 = ps_b.tile([128, 8, 128], BF, name="pv4", tag="ps_b")
                    for sc in range(8):
                        nc.tensor.transpose(
                            pv4[:, sc, :], vts[:, sc * 128:(sc + 1) * 128], ident_sb
                        )
                    # interleave into vt: head pair (2*half, 2*half+1)
                    v2 = vt[:, g * 8:(g + 1) * 8, :].rearrange(
                        "p k (a c) -> p k a c", c=65
                    )[:, :, 2 * half:2 * half + 2, 0:64]
                    s2 = pv4.rearrange("p k (a c) -> p k a c", c=64)
                    nc.vector.tensor_copy(out=v2, in_=s2)

        def attention(qb, r):
            """Causal attention for query block qb, head pair r.

            The PV matmul runs two key-blocks behind scores/exp so the PE
            queue (in-order) never waits on the ACT exp latency."""
            qsl = slice(qb * TCH, (qb + 1) * TCH)
            nkc = 8 * qb + 8

            def score_exp(kc, h):
                off = max(0, (kc - 8 * qb) * 128)
                nv = TCH - off
                hb = (h % 2) * 64
                c = h // 2
                ps = ps_a.tile([128, TCH], FP, name="ps", tag="ps_a")
                q0 = qb * TCH
                for c0, c1 in ((off, 512), (max(off, 512), TCH)):
                    if c0 >= c1:
                        continue
                    nc.tensor.matmul(
                        ps[:, c0:c1],
                        lhsT=kT[c][hb:hb + 64, kc * 128:(kc + 1) * 128],
                        rhs=qT[c][hb:hb + 64, q0 + c0:q0 + c1],
                        start=True,
                        stop=True,
                    )
                p = p_pool.tile([128, TCH], BF, name="p", tag="p")
                nc.scalar.activation(p[:, off:TCH], ps[:, off:TCH], EXP, scale=SCALE)
                if kc >= 8 * qb:  # diagonal band: triangular mask
                    eng = nc.gpsimd if "poolmask" in opts else nc.vector
                    eng.tensor_mul(
                        out=p[:, off:off + 128],
                        in0=p[:, off:off + 128],
                        in1=tri_sb,
                    )
                return p

            hs = (2 * r, 2 * r + 1)
            # pv accumulators split into column halves (one PSUM bank each)
            # so each half frees independently with a short normalize chain.
            HB = TCH // 2
            pv = {}
            for h in hs:
                for lh in range(2):
                    pv[h, lh] = ps_b.tile([65, HB], FP, name=f"ppv{h}{lh}", tag="ps_b")
            # last key block contributing to the lo half (cols [0, HB))
            lo_stop = 8 * qb + HB // 128 - 1

            def pv_mm(kc, ps_tiles):
                off = max(0, (kc - 8 * qb) * 128)
                for h in hs:
                    if off < HB:
                        nc.tensor.matmul(
                            pv[h, 0][:, off:HB],
                            lhsT=vt[:, kc, 65 * h:65 * h + 65],
                            rhs=ps_tiles[h][:, off:HB],
                            start=(kc == 0),
                            stop=(kc == lo_stop),
                        )
                    o2 = max(off, HB)
                    nc.tensor.matmul(
                        pv[h, 1][:, o2 - HB:HB],
                        lhsT=vt[:, kc, 65 * h:65 * h + 65],
                        rhs=ps_tiles[h][:, o2:TCH],
                        start=(kc == 0),
                        stop=(kc == nkc - 1),
                    )

            pk = {}
            for kc in range(nkc):
                pk[kc] = {h: score_exp(kc, h) for h in hs}
                if kc >= 2:
                    pv_mm(kc - 2, pk.pop(kc - 2))
            for kc in (nkc - 2, nkc - 1):
                if kc >= 0:
                    pv_mm(kc, pk.pop(kc))
            # normalize: ctx rows 0..63 per head, denominator row 64.
            # recip (DVE) -> partition broadcast (Pool) -> downcast mul
            # (DVE); no PE involvement so the following matmuls stream.
            for lh in range(2):
                for h in hs:
                    hb = (h % 2) * 64
                    c = h // 2
                    col = slice(qb * TCH + lh * HB, qb * TCH + (lh + 1) * HB)
                    # stage the denominator row to SBUF first (the custom
                    # DVE recip misreads PSUM on HW; DVE/ACT alternating)
                    den = nrm_pool.tile([1, HB], FP, name="den", tag="den")
                    nc.vector.tensor_copy(out=R(den), in_=pv[h, lh][64:65, :])
                    if "bcast" in opts:
                        rcp = nrm_pool.tile([1, HB], FP, name="rcp", tag="rcp")
                        nc.vector.reciprocal_approx_fast(out=rcp, in_=den)
                        rcb = nrm_pool.tile([64, HB], FP, name="rcb", tag="rcb")
                        nc.gpsimd.partition_broadcast(rcb, rcp, channels=64)
                    else:
                        dcb = ps_a.tile([64, HB], FP, name="dcb", tag="ps_a")
                        nc.tensor.matmul(
                            dcb, lhsT=R(onesr_sb), rhs=R(den), start=True, stop=True
                        )
                        rcb = nrm_pool.tile([64, HB], FP, name="rcb", tag="rcb")
                        nc.vector.reciprocal(rcb, dcb)
                    nc.vector.tensor_mul(
                        out=ctxT[c][hb:hb + 64, col], in0=pv[h, lh][0:64, :], in1=rcb
                    )

        def proj(qb):
            """Projection for query block qb's token range."""
            for half in range(2):
                obuf = ob_pool.tile([128, 4, D], BF, name="obuf", tag="ob")
                for j in range(4):
                    tb = 8 * qb + 4 * half + j
                    po = ps_a.tile([128, D], FP, name="po", tag="ps_a")
                    for c in range(2):
                        for hh in range(2):
                            nc.tensor.matmul(
                                po[:, hh * 512:(hh + 1) * 512],
                                lhsT=ctxT[c][:, tb * 128:(tb + 1) * 128],
                                rhs=wproj_sb[:, c, hh * 512:(hh + 1) * 512],
                                start=(c == 0),
                                stop=(c == 1),
                            )
                    # split the psum drain across DVE and ACT so the psum
                    # slot frees before the next-but-one matmul needs it
                    nc.vector.tensor_copy(out=obuf[:, j, 0:512], in_=po[:, 0:512])
                    nc.scalar.copy(obuf[:, j, 512:D], po[:, 512:D])
                if qb == 1 and half == 1:
                    # tail: per-block stores, last two split into col halves
                    for seg in range(4):
                        t0 = (8 * qb + 4 * half + seg) * 128
                        if seg < 2:
                            nc.sync.dma_start(out_d[t0:t0 + 128, :], obuf[:, seg, :])
                        else:
                            for ch in range(2):
                                nc.sync.dma_start(
                                    out_d[t0:t0 + 128, ch * 512:(ch + 1) * 512],
                                    obuf[:, seg, ch * 512:(ch + 1) * 512],
                                )
                else:
                    t0 = (8 * qb + 4 * half) * 128
                    dst = out_d[t0:t0 + 512, :].rearrange("(k p) f -> p k f", p=128)
                    nc.sync.dma_start(dst, obuf)

        def whole():
            if phases == "s1":
                for g in range(NG):
                    stage1(g, 0)
                    stage1(g, 1)
                return
            # Interleaving chosen so every attention round and every
            # normalize chain has dense PE work in front of it.
            stage1(0, 0)
            stage1(0, 1)
            attention(0, 0)
            stage1(1, 0)
            attention(0, 1)
            proj(0)
            stage1(1, 1)
            attention(1, 0)
            attention(1, 1)
            proj(1)
            if dbg:
                for c in range(2):
                    nc.sync.dma_start(qt_dbg[c], qT[c])
                    nc.sync.dma_start(kt_dbg[c], kT[c])
                    nc.sync.dma_start(ctx_dbg[c], ctxT[c])
                nc.sync.dma_start(vt_dbg, vt)

        if loop_n == 1:
            whole()
        else:
            with tc.For_i(0, loop_n, 1):
                whole()

    nc.compile()
    return nc


@functools.lru_cache(maxsize=4)
def _get_program(loop_n=1, phases="all", opts="bcast"):
    return _build_program(loop_n, phases, opts)


def _host_inputs(x, w_qkv, w_proj):
    """Build the 8 per-core input maps from the full problem inputs."""
    from ml_dtypes import bfloat16

    x = np.asarray(x, dtype=np.float32)
    w_qkv = np.asarray(w_qkv, dtype=np.float32)
    w_proj = np.asarray(w_proj, dtype=np.float32)

    # x transposed per batch: (KC, 128, S) bf16
    xt = {
        b: np.ascontiguousarray(x[b].T).reshape(KC, 128, S).astype(bfloat16)
        for b in range(B)
    }

    # RoPE tables, transposed + pair-replicated; sin is sign-folded.
    inv_freq = 1.0 / (ROPE_BASE ** (np.arange(0, Dh, 2, dtype=np.float32) / Dh))
    tpos = np.arange(S, dtype=np.float32)
    freqs = np.outer(tpos, inv_freq)                      # (S, 32)
    emb = np.concatenate([freqs, freqs], axis=-1)         # (S, 64)
    cosT = np.cos(emb).T.astype(np.float32)               # (64, S)
    sinT = np.sin(emb).T.astype(np.float32)
    sinT_f = sinT.copy()
    sinT_f[:32] *= -1.0                                   # fold rotate_half sign
    cos_full = np.ascontiguousarray(np.tile(cosT, (2, 1))).astype(bfloat16)
    sin_full = np.ascontiguousarray(np.tile(sinT_f, (2, 1))).astype(bfloat16)

    r = np.arange(128)
    tri = (r[None, :] >= r[:, None]).astype(bfloat16)     # tri[r, c] = c >= r

    wq = w_qkv[:, 0:D]
    wk = w_qkv[:, D:2 * D]
    wv = w_qkv[:, 2 * D:3 * D]

    in_maps = []
    for c in range(N_CORES):
        b, hq = divmod(c, HPC)
        cols = np.r_[4 * hq * 64:(4 * hq + 4) * 64]
        wqk_c = np.concatenate(
            [wq[:, cols], wk[:, cols], wv[:, cols]], axis=1
        )  # (D, 768)
        in_maps.append({
            "xt": xt[b],
            "wqk": np.ascontiguousarray(wqk_c).reshape(KC, 128, 768).astype(bfloat16),
            "wproj": np.ascontiguousarray(
                w_proj[4 * hq * 64:(4 * hq + 4) * 64, :]
            ).reshape(2, 128, D).astype(bfloat16),
            "cost": cos_full,
            "sint": sin_full,
            "tri": tri,
            "onesc": np.ones((128, NKB, 65 * HPC), dtype=bfloat16),
            "onesr": np.ones((1, 64), dtype=np.float32),
            "ident": np.eye(128, dtype=bfloat16),
        })
    return in_maps


_last_results = None


def kernel(x, w_qkv, w_proj):
    global _last_results
    from concourse.bass_utils import run_bass_kernel_spmd

    nc = _get_program()
    in_maps = _host_inputs(x, w_qkv, w_proj)
    trace = bool(int(os.environ.get("KERNEL_TRACE", "0")))
    kwargs = {}
    if trace:
        kwargs["trace"] = True
        kwargs["trace_cores"] = list(range(N_CORES))
    res = run_bass_kernel_spmd(nc, in_maps, core_ids=list(range(N_CORES)), **kwargs)
    _last_results = res
    acc = np.zeros((B, S, D), dtype=np.float32)
    for c, r in enumerate(res.results):
        acc[c // HPC] += r["out"].astype(np.float32)
    return acc

